# revision 23
# baseline (speedup 1.0000x reference)
"""BDS-vote (bidirectional NNF patch voting) Trainium2 kernel, v3.

Formulation
-----------
Both passes add 3x3 ref patches into 3x3 output neighborhoods:
  pass0 (w=ws): for grid center c:   guide[c+d] += ws * ref[nnf_sr[c]+d]
  pass1 (w=wr): for ref  center q:   guide[nnf_rs[q]+d] += wr * ref[q+d]
with per-pixel validity <=> zero padding of the gather source.

"Triple-row" layout T[y', c] = (ref[y'-2, c-1], ref[y'-1, c-1],
ref[y', c-1]) zero-padded: the full 3x3 patch at center (uy, ux) is the 3
consecutive triples (uy+1; ux..ux+2) -- one contiguous 2304B element.

Each item is one (dst center p, src center u) pair: a 2304B dma_gather
from T (ws-prescaled f32) and a 2304B dma_scatter_add into Tacc (same
triple layout) at p. pass1 chunks are scaled x2 (completeness) on the
Activation engine between gather and scatter (single shared table).
Output row y then reads plane i of Tacc row y+2-i, cols x+1,
PE-transposed (PSUM-accumulated) to channel-major, * 1/weight.

v3 vs v2: SPMD cross-core chunk padding uses -1 indices (descriptor-free
on HW) with per-core real counts loaded into Pool registers; single
table (no wr-scaled copy); invw input is [SLAB, W] broadcast on-device;
assembly reads each Tacc row once (rolling window, PSUM accumulation);
inputs packed into two blob tensors.

Sharding: core k owns output rows [64k, 64k+64); items bucketed by
destination row slab (halo 1 row) on the host; no collectives. SPMD: one
program; per-core index arrays; cross-core-max chunk plan with -1 pads.
"""

import os
import sys

for p in ("/opt/trn_rl_repo",):
    if p not in sys.path:
        sys.path.insert(0, p)

import numpy as np

# ---------------------------------------------------------------- params
C = 64
H = 512
W = 512
HW = H * W
NCORES = 8
SLAB = H // NCORES              # 64 output rows per core
COMPLETENESS = 2.0
WS = 1.0 / HW
WR = COMPLETENESS / HW

XT = 514                        # triple cols (pixel col = c-1)
YT = 514                        # triple rows y' (center row = y'-1)
TACC_TRASH = 1                  # trash rows per region
TACC_REG = 2                    # scatter regions; separate DRAM tensors so
                                # their scatter chains pipeline independently
REG_ROWS = 33                   # real rows per region (4*17 >= 66)
REG_TRI = (TACC_TRASH + REG_ROWS) * XT          # triples per region tensor
NWIN = 9                        # T gather windows of 63 rows
WINROWS = 63
T_TRI = YT * XT + 8             # flat triples in T (+slack)
SEG_CH = 1024                   # max items per SWDGE instruction
AHEAD = 6                       # gathers issued ahead of matching scatter

F32 = np.float32
I16 = np.int16
I32 = np.int32


def _wrap16(idx):
    n = idx.shape[0]
    assert n % 16 == 0
    assert idx.min() >= -1 and idx.max() < 32768, (idx.min(), idx.max())
    blk = idx.reshape(n // 16, 16).T.astype(I16)
    return np.tile(blk, (8, 1))


def _layerize(g, s, width):
    """Split items into layers whose scatter windows [s, s+width) are
    pairwise disjoint within a layer.  Greedy interval coloring on the
    sorted starts: layer count == max overlap depth (minimal)."""
    if len(s) == 0:
        return []
    o = np.argsort(s, kind="stable")
    g, s = g[o], s[o]
    ends = []          # last end per layer
    lay = np.empty(len(s), np.int64)
    for j in range(len(s)):
        sj = s[j]
        for li in range(len(ends)):
            if ends[li] <= sj:
                ends[li] = sj + width
                lay[j] = li
                break
        else:
            lay[j] = len(ends)
            ends.append(sj + width)
    return [(g[lay == li], s[lay == li]) for li in range(len(ends))]


# ---------------------------------------------------------------- host prep
def _weight_invw(nnf_sr, nnf_rs):
    gy, gx = np.meshgrid(np.arange(H, dtype=np.int64),
                         np.arange(W, dtype=np.int64), indexing="ij")
    gy, gx = gy.ravel(), gx.ravel()
    ty0 = np.concatenate([gy, nnf_rs[..., 0].ravel().astype(np.int64)])
    tx0 = np.concatenate([gx, nnf_rs[..., 1].ravel().astype(np.int64)])
    qy0 = np.concatenate([nnf_sr[..., 0].ravel().astype(np.int64), gy])
    qx0 = np.concatenate([nnf_sr[..., 1].ravel().astype(np.int64), gx])
    wvals = np.concatenate([np.full(HW, WS), np.full(HW, WR)])
    weight = np.zeros(HW, np.float64)
    for dy in (-1, 0, 1):
        tr, qr = ty0 + dy, qy0 + dy
        vrow = (tr >= 0) & (tr < H) & (qr >= 0) & (qr < H)
        for dx in (-1, 0, 1):
            tc_, qc = tx0 + dx, qx0 + dx
            v = vrow & (tc_ >= 0) & (tc_ < W) & (qc >= 0) & (qc < W)
            weight += np.bincount(tr[v] * W + tc_[v], weights=wvals[v],
                                  minlength=HW)
    weight = weight.astype(F32)
    return (1.0 / np.where(weight == 0, 1.0, weight)).astype(F32).reshape(H, W)


def _prep(ref, nnf_sr, nnf_rs):
    """Build host arrays: T (ws-prescaled), invw slabs, index streams, plan."""
    invw = _weight_invw(nnf_sr, nnf_rs)

    refp = np.asarray(ref, F32).transpose(1, 2, 0)          # [H, W, C]
    Tm = np.zeros((YT, XT, 3, C), F32)
    for i in range(3):
        Tm[2 - i:514 - i, 1:513, i, :] = refp
    t_ws = np.zeros((T_TRI, 256), np.float16)
    t_ws[:YT * XT, :192] = Tm.reshape(YT * XT, 3 * C)
    del Tm

    # ---- items: (dst p, src u) for both passes
    gy, gx = np.meshgrid(np.arange(H, dtype=np.int64),
                         np.arange(W, dtype=np.int64), indexing="ij")
    gy, gx = gy.ravel(), gx.ravel()
    pys = [gy, nnf_rs[..., 0].ravel().astype(np.int64)]
    pxs = [gx, nnf_rs[..., 1].ravel().astype(np.int64)]
    uys = [nnf_sr[..., 0].ravel().astype(np.int64), gy]
    uxs = [nnf_sr[..., 1].ravel().astype(np.int64), gx]

    streams = {}   # (core, pass, half, win) -> [(g, s) per layer]
    for k in range(NCORES):
        for ps in range(2):
            py, px, uy, ux = pys[ps], pxs[ps], uys[ps], uxs[ps]
            sel = (py >= 64 * k - 1) & (py < 64 * k + SLAB + 1)
            lr = py[sel] + 1 - 64 * k                   # [0, 66)
            reg = lr // REG_ROWS
            lrr = lr - reg * REG_ROWS
            sidx = (TACC_TRASH + lrr) * XT + px[sel]
            gtri = (uy[sel] + 1) * XT + ux[sel]
            win = (uy[sel] + 1) // WINROWS
            for h in range(TACC_REG):
                for w in range(NWIN):
                    m = (reg == h) & (win == w)
                    gl = gtri[m] - w * WINROWS * XT
                    streams[(k, ps, h, w)] = _layerize(gl.copy(),
                                                       sidx[m].copy(), 3)

    # chunk list per region (FIFO within region), then round-robin emit so
    # consecutive scatters always target distinct regions (safe DMA overlap)
    reg_chunks = {h: [] for h in range(TACC_REG)}
    for ps in range(2):
        for h in range(TACC_REG):
            for w in range(NWIN):
                lays = [streams[(k, ps, h, w)] for k in range(NCORES)]
                nlayers = max((len(x) for x in lays), default=0)
                for li in range(nlayers):
                    nmax = max(len(x[li][0]) if li < len(x) else 0 for x in lays)
                    if nmax == 0:
                        continue
                    nfull, rem = divmod(nmax, SEG_CH)
                    sizes = [SEG_CH] * nfull + \
                        ([-(-rem // 16) * 16] if rem else [])
                    total = sum(sizes)
                    gs, ss, base_ns = [], [], []
                    for k in range(NCORES):
                        if li < len(lays[k]):
                            g, s = lays[k][li]
                        else:
                            g = s = np.zeros(0, np.int64)
                        # pad to a multiple of 16 with trash items (real
                        # descriptors into trash rows), then -1 (skipped)
                        n_real = g.shape[0]
                        n16 = -(-max(n_real, 16) // 16) * 16
                        ntrash = n16 - n_real
                        j = np.arange(ntrash, dtype=np.int64)
                        npad = total - n16
                        gs.append(np.concatenate(
                            [g, np.zeros(ntrash, np.int64),
                             np.full(npad, -1, np.int64)]))
                        ss.append(np.concatenate(
                            [s, 3 * j, np.full(npad, -1, np.int64)]))
                        base_ns.append(n16)
                    off = 0
                    for L in sizes:
                        cnts = [min(max(n - off, 0), L) for n in base_ns]
                        # every chunk keeps >= 16 live lanes per core: turn
                        # leading pad lanes of empty chunks into trash items
                        for k in range(NCORES):
                            if cnts[k] < 16:
                                need = 16 - cnts[k]
                                st_ = cnts[k]
                                j = np.arange(need, dtype=np.int64)
                                gs[k][off + st_:off + st_ + need] = 0
                                ss[k][off + st_:off + st_ + need] = \
                                    3 * (st_ + j)
                                cnts[k] = 16
                        reg_chunks[h].append(
                            (ps, h, w, L,
                             [x[off:off + L] for x in gs],
                             [x[off:off + L] for x in ss],
                             cnts))
                        off += L

    # emission: front-load the lowest unfinished region (every other slot,
    # alternating with a rotation of the others) so regions complete in
    # order 0,1,2,3 and the final assembly overlaps the tail of the
    # scatter phase.  Consecutive scatters always target distinct regions.
    plan = []          # (pass, reg, win, col_off, L)
    gcols = {k: [] for k in range(NCORES)}
    scols = {k: [] for k in range(NCORES)}
    cnt_rows = {k: [] for k in range(NCORES)}
    col = 0
    heads = {h: 0 for h in range(TACC_REG)}

    def emit(h):
        nonlocal col
        ps, hh, w, L, gs, ss, cnts = reg_chunks[h][heads[h]]
        heads[h] += 1
        for k in range(NCORES):
            gcols[k].append(gs[k])
            scols[k].append(ss[k])
            cnt_rows[k].append(cnts[k])
        plan.append((ps, hh, w, col, L))
        col += L // 16

    rot = 1
    last = -1
    while True:
        live = [h for h in range(TACC_REG) if heads[h] < len(reg_chunks[h])]
        if not live:
            break
        front = live[0]
        if front != last:
            emit(front)
            last = front
            continue
        others = [h for h in live if h != last] or live
        pick = min(others, key=lambda h: ((h - rot) % TACC_REG, h))
        rot += 1
        emit(pick)
        last = pick

    nplan = len(plan)
    per_core = []
    for k in range(NCORES):
        p1g = _wrap16(np.concatenate(gcols[k]))
        p1s = _wrap16(np.concatenate(scols[k]))
        blob_i16 = np.ascontiguousarray(
            np.concatenate([p1g, p1s], axis=1))          # [128, 2*cols]
        cnts = np.asarray(cnt_rows[k], I32)
        invw_k = np.ascontiguousarray(invw[64 * k:64 * k + SLAB]) * F32(WS)
        blob_f32 = np.concatenate(
            [invw_k.reshape(-1),
             cnts.view(F32),
             np.zeros((-len(cnt_rows[k])) % 4, F32)])
        per_core.append({"blob_i16": blob_i16, "blob_f32": blob_f32})
    return plan, per_core, t_ws, nplan


# ---------------------------------------------------------------- emulator
def _emulate(plan, per_core, t_ws):
    """Numpy emulation of the device program."""
    outs = []
    t_flat = np.asarray(t_ws, F32).reshape(-1)
    for k in range(NCORES):
        d = per_core[k]
        cols = d["blob_i16"].shape[1] // 2
        p1g = d["blob_i16"][:, :cols]
        p1s = d["blob_i16"][:, cols:]
        nplan = len(plan)
        invw = d["blob_f32"][:SLAB * W].reshape(SLAB, W)
        cnts = d["blob_f32"][SLAB * W:SLAB * W + nplan].view(I32)
        tacc = [np.zeros(REG_TRI * 256, F32) for _ in range(TACC_REG)]
        for ci, (ps, h, w, coff, L) in enumerate(plan):
            gi = p1g[:16, coff:coff + L // 16].T.reshape(-1).astype(np.int64)
            si = p1s[:16, coff:coff + L // 16].T.reshape(-1).astype(np.int64)
            n = int(cnts[ci])
            gi, si = gi[:n], si[:n]
            assert (gi >= 0).all() and (si >= 0).all()
            src_base = w * WINROWS * XT * 256
            scale = COMPLETENESS if ps == 1 else 1.0
            src = t_flat[src_base + gi[:, None] * 256
                         + np.arange(768)[None, :]] * scale
            np.add.at(tacc[h], si[:, None] * 256
                      + np.arange(768)[None, :], src)
        tacc = [t.reshape(TACC_TRASH + REG_ROWS, XT, 4, C) for t in tacc]

        out = np.zeros((C, SLAB, W), F32)
        for ly in range(SLAB):
            acc = np.zeros((C, W), F32)
            for i in range(3):
                lr = ly + 2 - i
                hh = lr // REG_ROWS
                acc += tacc[hh][TACC_TRASH + (lr - hh * REG_ROWS),
                                1:513, i, :].T
            out[:, ly, :] = acc * invw[ly][None, :]
        outs.append(out)
    return np.concatenate(outs, axis=1)


# ---------------------------------------------------------------- program
def _build(plan, i16cols, nf32):
    from concourse import bacc, bass, tile
    from concourse import mybir
    from concourse.masks import make_identity

    f32 = mybir.dt.float32
    f16 = mybir.dt.float16
    i16 = mybir.dt.int16
    i32 = mybir.dt.int32

    nplan = len(plan)
    cols = i16cols // 2

    nc = bacc.Bacc("TRN2", target_bir_lowering=False, debug=False,
                   num_devices=NCORES, num_swdge_queues=2)

    tws = nc.dram_tensor("tws", [T_TRI, 256], f16, kind="ExternalInput")
    blob16 = nc.dram_tensor("blob16", [128, i16cols], i16, kind="ExternalInput")
    blobf = nc.dram_tensor("blobf", [1, nf32], f32, kind="ExternalInput")
    taccs = [nc.dram_tensor(f"tacc{h}", [REG_TRI + 8, 256], f16)
             for h in range(TACC_REG)]
    out = nc.dram_tensor("out", [C, SLAB, W], f32, kind="ExternalOutput")

    with tile.TileContext(nc) as tc:
        with (
            tc.tile_pool(name="idxp", bufs=1) as idxp,
            tc.tile_pool(name="zp", bufs=1) as zp,
            tc.tile_pool(name="stgp", bufs=8) as stgp,
            tc.tile_pool(name="tap", bufs=6) as tap,
            tc.tile_pool(name="fin", bufs=3) as fin,
            tc.tile_pool(name="bcp", bufs=3) as bcp,
            tc.tile_pool(name="pp", bufs=8, space="PSUM") as pp,
        ):
            t_p1 = idxp.tile([128, i16cols], i16, tag="p1")
            nc.sync.dma_start(out=t_p1[:], in_=blob16[:])
            t_cnt = idxp.tile([1, nplan], i32, tag="cnt")
            nc.sync.dma_start(
                out=t_cnt[:],
                in_=bass.AP(blobf, SLAB * W, [(1, nplan)]).bitcast(i32))
            ident = idxp.tile([128, 128], f16, tag="id")
            make_identity(nc, ident[:])

            # zero tacc regions (interleaved so the copies pipeline)
            zt = zp.tile([128, 4096], f16, tag="z")
            nc.vector.memset(zt[:], 0.0)
            ZROWS = 2048
            zoffs = [0] * TACC_REG
            live = True
            while live:
                live = False
                for h in range(TACC_REG):
                    r = zoffs[h]
                    if r >= REG_TRI + 8:
                        continue
                    rows = min(ZROWS, REG_TRI + 8 - r)
                    nc.sync.dma_start(out=taccs[h][r:r + rows, :],
                                      in_=zt[:, :rows * 256 // 128])
                    zoffs[h] = r + rows
                    live = True

            # ---- both passes: gather T patches -> scatter-add into tacc
            # (software-pipelined: gathers issued AHEAD of the matching
            #  scatter so Pool desc-gen overlaps in-flight DMAs)
            pend = []

            def issue_gather(ci):
                ps, h, w, coff, L = plan[ci]
                src_base = w * WINROWS * XT
                cnt = min(WINROWS * XT + 1040, T_TRI - src_base - 2)
                src_ap = bass.AP(tws, src_base * 256, [(256, cnt), (1, 768)])
                st = stgp.tile([128, (L + 127) // 128, 768], f16, tag="stg")
                cv = nc.gpsimd.value_load(t_cnt[0:1, ci:ci + 1])
                nc.gpsimd.dma_gather(st[:], src_ap,
                                     t_p1[:, coff:coff + L // 16],
                                     L, cv, 768, elem_step=256, queue_num=0)
                if ps == 1:
                    nc.scalar.mul(st[:], st[:], COMPLETENESS)
                pend.append((ci, st, cv))

            def issue_scatter():
                ci, st, cv = pend.pop(0)
                ps, h, w, coff, L = plan[ci]
                dst_ap = bass.AP(taccs[h], 0, [(256, REG_TRI + 4), (1, 768)])
                nc.gpsimd.dma_scatter_add(dst_ap, st[:],
                                          t_p1[:, cols + coff:cols + coff + L // 16],
                                          L, cv, 768, elem_step=256,
                                          queue_num=1)

            for ci in range(nplan):
                issue_gather(ci)
                if len(pend) > AHEAD:
                    issue_scatter()
            while pend:
                issue_scatter()

            # ---- final assembly per output row (rolling Tacc-row window)
            row_tiles = {}

            def load_row(lr):
                hh = lr // REG_ROWS
                pr = TACC_TRASH + (lr - hh * REG_ROWS)
                t_row = tap.tile([128, 4, 256], f16, tag="trow")
                src = bass.AP(taccs[hh], (pr * XT + 1) * 256,
                              [(256, 128), (128 * 256, 4), (1, 256)])
                nc.sync.dma_start(out=t_row[:], in_=src)
                row_tiles[lr] = t_row

            load_row(0)
            load_row(1)
            for ly in range(SLAB):
                load_row(ly + 2)
                t_iw1 = bcp.tile([1, W], f32, tag="iw1")
                nc.sync.dma_start(out=t_iw1[:],
                                  in_=bass.AP(blobf, ly * W, [(W, 1), (1, W)]))
                t_iwb = bcp.tile([64, W], f32, tag="iwb")
                nc.gpsimd.partition_broadcast(t_iwb[:], t_iw1[0:1, :])
                t_acc = fin.tile([64, W], f32, tag="acc")
                for q in range(4):
                    psu = pp.tile([64, 128], f32, tag="ps")
                    for i in range(3):
                        t_row = row_tiles[ly + 2 - i]
                        nc.tensor.matmul(psu[:],
                                         t_row[:, q, 64 * i:64 * (i + 1)],
                                         ident[:, :128],
                                         start=(i == 0), stop=(i == 2))
                    nc.vector.tensor_tensor(
                        out=t_acc[:, 128 * q:128 * (q + 1)],
                        in0=psu[:],
                        in1=t_iwb[:, 128 * q:128 * (q + 1)],
                        op=mybir.AluOpType.mult)
                del row_tiles[ly]
                nc.sync.dma_start(out=out[:, ly, :], in_=t_acc[:])

    nc.compile()
    return nc


LAST_RUN_INFO = {}


def _run_spmd(nc, in_maps, time_it=False):
    """SPMD runner; inputs staged per-device; outputs pre-staged, not donated."""
    import jax
    from jax.experimental.shard_map import shard_map
    from jax.sharding import Mesh, PartitionSpec, NamedSharding
    from concourse import bass2jax, mybir

    n_cores = len(in_maps)
    bass2jax.install_neuronx_cc_hook()
    if nc.dbg_addr is not None:
        assert not nc.dbg_callbacks
        in_maps = [{**m, nc.dbg_addr.name: np.zeros((1, 2), np.uint32)}
                   for m in in_maps]
    partition_name = nc.partition_id_tensor.name if nc.partition_id_tensor else None

    in_names, out_names, out_avals = [], [], []
    for alloc in nc.m.functions[0].allocations:
        if not isinstance(alloc, mybir.MemoryLocationSet):
            continue
        name = alloc.memorylocations[0].name
        if alloc.kind == "ExternalInput":
            if name != partition_name:
                in_names.append(name)
        elif alloc.kind == "ExternalOutput":
            out_names.append(name)
            out_avals.append(jax.core.ShapedArray(
                tuple(alloc.tensor_shape), mybir.dt.np(alloc.dtype)))
    n_params = len(in_names)
    all_names = in_names + out_names
    if partition_name is not None:
        all_names = all_names + [partition_name]

    def _body(*args):
        operands = list(args)
        if partition_name is not None:
            operands.append(bass2jax.partition_id_tensor())
        outs = bass2jax._bass_exec_p.bind(
            *operands,
            out_avals=tuple(out_avals),
            in_names=tuple(all_names),
            out_names=tuple(out_names),
            lowering_input_output_aliases=(),
            sim_require_finite=True,
            sim_require_nnan=True,
            nc=nc,
        )
        return tuple(outs)

    devices = jax.devices()[:n_cores]
    mesh = Mesh(np.array(devices), ("core",))
    spec = PartitionSpec("core")
    sharding = NamedSharding(mesh, spec)

    def gput(per_core):
        shape = (n_cores * per_core[0].shape[0], *per_core[0].shape[1:])
        parts = [jax.device_put(per_core[c], devices[c]) for c in range(n_cores)]
        return jax.make_array_from_single_device_arrays(shape, sharding, parts)

    global_ins = [gput([np.asarray(m[name]) for m in in_maps])
                  for name in in_names]
    zero_ins = [gput([np.zeros(a.shape, a.dtype) for _ in range(n_cores)])
                for a in out_avals]
    sharded = jax.jit(
        shard_map(_body, mesh=mesh,
                  in_specs=(spec,) * (n_params + len(out_names)),
                  out_specs=(spec,) * len(out_names), check_rep=False),
        keep_unused=True)

    args = (*global_ins, *zero_ins)
    try:
        # AOT-compile once: calling the Compiled object skips per-call
        # tracing/cache lookup, which is a large share of dispatch time.
        sharded = sharded.lower(*args).compile()
    except Exception:
        pass
    out_arrs = sharded(*args)
    exec_ns = None
    if time_it:
        # Per-execution device time. Single blocking calls over the axon
        # tunnel are dominated by ~100ms of RPC dispatch latency, so the
        # kernel is timed in steady state: submit KREP executions
        # back-to-back (they serialize on-device) and divide the span.
        import time
        jax.block_until_ready(out_arrs)
        KREP = 200
        best = None
        for _ in range(3):
            t0 = time.perf_counter()
            outs = [sharded(*args) for _ in range(KREP)]
            jax.block_until_ready(outs)
            dt = (time.perf_counter() - t0) / KREP
            best = dt if best is None else min(best, dt)
        exec_ns = int(best * 1e9)
    results = []
    for c in range(n_cores):
        d = {}
        for i, name in enumerate(out_names):
            shards = sorted(out_arrs[i].addressable_shards,
                            key=lambda s: s.index[0].start or 0)
            d[name] = np.asarray(shards[c].data)
        results.append(d)
    return results, exec_ns


# ---------------------------------------------------------------- entry
def kernel(ref: np.ndarray, nnf_sr: np.ndarray, nnf_rs: np.ndarray) -> np.ndarray:
    assert ref.shape == (C, H, W) and nnf_sr.shape == (H, W, 2)
    plan, per_core, t_ws, nplan = _prep(np.asarray(ref, F32),
                                        np.asarray(nnf_sr), np.asarray(nnf_rs))

    if int(os.environ.get("KERNEL_EMULATE", "0")):
        return _emulate(plan, per_core, t_ws).astype(np.asarray(ref).dtype)

    i16cols = per_core[0]["blob_i16"].shape[1]
    nf32 = per_core[0]["blob_f32"].shape[0]
    nc = _build(plan, i16cols, nf32)
    in_maps = []
    for k in range(NCORES):
        d = per_core[k]
        in_maps.append({"tws": t_ws, "blob16": d["blob_i16"],
                        "blobf": d["blob_f32"].reshape(1, -1)})
    time_it = bool(int(os.environ.get("KERNEL_TIME", "0")))
    results, exec_ns = _run_spmd(nc, in_maps, time_it=time_it)
    LAST_RUN_INFO.clear()
    LAST_RUN_INFO["exec_time_ns"] = exec_ns
    slabs = [results[k]["out"] for k in range(NCORES)]
    return np.ascontiguousarray(np.concatenate(slabs, axis=1)).astype(ref.dtype)


if __name__ == "__main__":
    rng = np.random.default_rng(0)
    ref = rng.standard_normal((C, H, W)).astype(F32)
    nsr = rng.integers(0, 512, (H, W, 2)).astype(np.int32)
    nrs = rng.integers(0, 512, (H, W, 2)).astype(np.int32)
    out = kernel(ref, nsr, nrs)
    print(out.shape, out.dtype, LAST_RUN_INFO)
